# revision 3
# baseline (speedup 1.0000x reference)
"""Pixel-unshuffle (down_scale=2) Trainium2 Bass kernel.

Full input put: (16, 64, 512, 512) f32 -> output (16, 256, 256, 256) f32,
out[n, 4g + 2y + x, i, j] = put[n, g, 2i + y, 2j + x].

Sharding: batch dim split across 8 NeuronCores (2 batches per core); the
permutation is local to each (n, g) plane so no communication is needed.

Per-core dataflow (pure data movement, memory-bound; 128 MiB in + 128 MiB
out per core). Each iteration handles g=4 input planes:
  - one 4 MiB contiguous load into SBUF laid out so partition q holds input
    rows 4q..4q+3 of each plane (plain row-major reshape to (128, 2048) per
    plane),
  - 4 strided DVE tensor_copies (one per output-channel offset c2=2y+x)
    deinterleave even/odd rows+columns into an out tile whose flat layout
    equals 16 contiguous output channel planes,
  - one 4 MiB store (2 KiB contiguous DRAM runs).
Tile double/triple-buffers (bufs=3) so loads, copies and stores of
different iterations overlap.

Loads and stores share ONE HWDGE ring (both issued on nc.sync). Within a
ring, descriptors drain FIFO per DMA, so HBM reads and writes alternate at
4 MiB granularity instead of mixing at ~4 KB packet granularity as with
the two-ring (sync+scalar) split. Directional probes showed read-only
~330 GB/s and write-only ~356 GB/s, while the packet-mixed 2-ring kernel
aggregates only ~325 GB/s; coarse alternation recovers ~3.5%
(same-session A/B: 1-ring 801-806 us vs 2-ring 829-835 us per pass).

Measured on 8 axon-tunneled trn2 cores (repeat-loop differencing): ~805 us
per pass, bit-exact vs the jax reference, BELOW the 2-ring contiguous
SBUF-bounce memcpy of the same bytes (~815-835 us) thanks to the
single-ring read/write alternation. The cost-model timeline simulator
predicts 749 us (the exact DMA roofline at its nominal 358 GB/s),
confirming the schedule is bubble-free; the residual ~7% is the read-path
ceiling (read-only probe: ~330 GB/s), invariant across DMA size (2-8
MiB) and descriptor run length (2-16 KiB) in A/B probes.

Further A/B attempts this session (all slower or incorrect, reverted):
L L S S paired issue with bufs=1 (782 us/pass steady-state but
intermittently corrupt output - race not caught without CoreSim, NOT
shipped); plane-block layouts r=8/16 (longer read runs 16-32 KiB help
reads in isolation - 350-375 us/pass read-only vs 402 - but force >=4 KiB
localized write runs which cost writes more than the read gain: full
kernels 844-894 us); any 2-ring split (packet-level read/write mixing,
815-844 us).
"""

import numpy as np

N_CORES = 8
N_FULL = 16  # full batch
N_PER_CORE = N_FULL // N_CORES  # 2
C_IN = 64
H = 512
W = 512
R = 2
HP = H // R  # 256
WP = W // R  # 256
C_OUT = C_IN * R * R  # 256

_CACHE = {}


def _build_module(
    copy_engines=("vector", "vector", "vector", "vector"),
    bufs=3,
    n_passes=1,
    g=4,
    alt_rings=False,
    single_ring=True,
):
    import concourse.bacc as bacc
    import concourse.mybir as mybir
    from concourse.tile import TileContext

    nc = bacc.Bacc("TRN2", target_bir_lowering=False, debug=False)
    x = nc.dram_tensor(
        "x", (N_PER_CORE, C_IN, H, W), mybir.dt.float32, kind="ExternalInput"
    )
    y = nc.dram_tensor(
        "y", (N_PER_CORE, C_OUT, HP, WP), mybir.dt.float32, kind="ExternalOutput"
    )

    def body(pool):
        for n in range(N_PER_CORE):
            for gg in range(C_IN // g):
                g0 = g * gg
                if single_ring:
                    load_eng, store_eng = nc.sync, nc.sync
                elif alt_rings and gg % 2 == 1:
                    load_eng, store_eng = nc.scalar, nc.sync
                else:
                    load_eng, store_eng = nc.sync, nc.scalar
                # ---- load: g planes, partition q <- rows 4q..4q+3 of each
                in_tile = pool.tile([128, g * 2048], mybir.dt.float32, name="in_tile")
                src = x[n, g0 : g0 + g].rearrange("g (q r) w -> q g (r w)", r=4)
                load_eng.dma_start(
                    out=in_tile.rearrange("p (g e) -> p g e", g=g), in_=src
                )

                # ---- deinterleave into output-plane layout
                out_tile = pool.tile(
                    [128, g * 2048], mybir.dt.float32, name="out_tile"
                )
                # in free dim: (g, hp, yy, w2, xx) sizes (g, 2, 2, 256, 2)
                v = in_tile.rearrange(
                    "p (g hp yy w2 xx) -> p yy xx g hp w2",
                    g=g, hp=2, yy=2, w2=256, xx=2,
                )
                # out free dim: (g, c, hp, w2) sizes (g, 4, 2, 256)
                o = out_tile.rearrange(
                    "p (g c hp w2) -> p c g hp w2", g=g, c=4, hp=2, w2=256
                )
                for yy in range(2):
                    for xx in range(2):
                        c2 = 2 * yy + xx
                        eng = copy_engines[c2]
                        if eng == "vector":
                            nc.vector.tensor_copy(out=o[:, c2], in_=v[:, yy, xx])
                        elif eng == "scalar":
                            nc.scalar.copy(out=o[:, c2], in_=v[:, yy, xx])
                        elif eng == "gpsimd":
                            nc.gpsimd.tensor_copy(out=o[:, c2], in_=v[:, yy, xx])
                        else:
                            raise ValueError(eng)

                # ---- store: 4g contiguous output channel planes
                dst = y[n, 4 * g0 : 4 * g0 + 4 * g].rearrange(
                    "(g c) (q hh) w -> q g c (hh w)", g=g, hh=2
                )
                store_eng.dma_start(
                    out=dst,
                    in_=out_tile.rearrange("p (g c e) -> p g c e", g=g, c=4),
                )

    with TileContext(nc) as tc:
        with tc.tile_pool(name="io", bufs=bufs) as pool:
            if n_passes == 1:
                body(pool)
            else:
                with tc.For_i(0, n_passes, 1):
                    body(pool)
    nc.finalize()
    return nc


def _get_module():
    key = "module"
    if key not in _CACHE:
        _CACHE[key] = _build_module()
    return _CACHE[key]


def _shard_inputs(put):
    put = np.ascontiguousarray(np.asarray(put, dtype=np.float32))
    return [
        {"x": put[i * N_PER_CORE : (i + 1) * N_PER_CORE]} for i in range(N_CORES)
    ]


def _unshard(per_core_ys):
    return np.concatenate(list(per_core_ys), axis=0)


def _run(put, trace=False):
    from concourse.bass_utils import run_bass_kernel_spmd

    nc = _get_module()
    in_maps = _shard_inputs(put)
    res = run_bass_kernel_spmd(
        nc, in_maps, core_ids=list(range(N_CORES)), trace=trace
    )
    out = _unshard([r["y"] for r in res.results])
    return out, res


def kernel(put, down_scale):
    r = int(down_scale)
    put = np.asarray(put)
    if r != R or put.shape != (N_FULL, C_IN, H, W):
        # generic fallback (correct for any shape, CPU)
        n, c, h, w = put.shape
        z = put.reshape(n, c, h // r, r, w // r, r)
        z = np.transpose(z, (0, 1, 3, 5, 2, 4))
        return np.ascontiguousarray(z.reshape(n, c * r * r, h // r, w // r))
    out, _ = _run(put, trace=False)
    return out

